# revision 5
# baseline (speedup 1.0000x reference)
"""Multi-head attention (N=4, L=2048, E=1024, H=16) on 8 Trainium2 cores.

Sharding: core c -> (batch n = c // 2, head-group g = c % 2).  Each core
computes, for its batch and its 8 heads (512 embed dims):
  qp_T/kp_T = (Wg q^T) in [d, tok] layout, vp in [tok, d] layout,
  S_T[k, q] = kp_T^T-contracted scores (two heads packed in the 128
  partitions via PE row tiling), exp via ACT with the 1/sqrt(1024) scale
  folded in, attn@v with a ones column appended to vp so the softmax
  denominator accumulates in the same PSUM tile, normalization via a
  1-partition PE replicate matmul + DVE multiply, then the output
  projection against Wo columns of this head group.
Host sums the two per-group partial outputs per batch and adds bo.

All matmuls run in float32r (TF32-like, ~1e-4 rel err, full PE rate).
"""

import numpy as np

import concourse.bacc as bacc
import concourse.mybir as mybir
import concourse.tile as tile
from concourse.bass import ds, ts
from concourse.bass_utils import run_bass_kernel_spmd

F32 = mybir.dt.float32
F32R = mybir.dt.float32r

E = 1024          # embed
H = 16            # heads (global)
D = 64            # head dim
L = 2048          # sequence length
NB = 4            # batch
GE = 512          # embed dims per head group (8 heads)
P = 128           # partitions
TB = L // 512     # 4 token blocks of 512
EC = E // P       # 8 embed chunks
DC = GE // P      # 4 d-chunks per group == head pairs
KT = L // P       # 16 key-token chunks

_CACHE = {}


def _build():
    nc = bacc.Bacc("TRN2", debug=False, enable_asserts=False, num_devices=8)

    xq = nc.dram_tensor("xq", [E, L], F32R, kind="ExternalInput").ap()
    xk = nc.dram_tensor("xk", [E, L], F32R, kind="ExternalInput").ap()
    xv = nc.dram_tensor("xv", [E, L], F32R, kind="ExternalInput").ap()
    wq = nc.dram_tensor("wq", [E, GE], F32R, kind="ExternalInput").ap()
    wk = nc.dram_tensor("wk", [E, GE], F32R, kind="ExternalInput").ap()
    wv = nc.dram_tensor("wv", [E, GE], F32R, kind="ExternalInput").ap()
    wo = nc.dram_tensor("wo", [GE, E], F32R, kind="ExternalInput").ap()
    bqk = nc.dram_tensor("bqk", [2, P, DC], F32, kind="ExternalInput").ap()
    bvr = nc.dram_tensor("bvr", [1, GE], F32R, kind="ExternalInput").ap()
    out = nc.dram_tensor("out", [L, E], F32, kind="ExternalOutput").ap()

    with tile.TileContext(nc) as tc:
        with tc.tile_pool(name="persist", bufs=1) as pp, \
             tc.tile_pool(name="stage", bufs=1, space="DRAM") as stage:
            # persistent SBUF
            vp = pp.tile([P, KT, 8, D + 1], F32R)        # vp_aug per head
            ao = pp.tile([P, DC, L], F32R)               # normalized attnout_T
            ones32 = pp.tile([1, P], F32)
            ones = pp.tile([1, P], F32R)
            # DRAM staging for q/k projections, [pair, 128, L]
            qs = stage.tile([DC, P, L], F32R)
            ks = stage.tile([DC, P, L], F32R)

            nc.gpsimd.memset(ones32[:], 1.0)
            nc.vector.tensor_copy(ones[:], ones32[:])

            # ---------------- phase 1: projections ----------------
            with tc.tile_pool(name="wpool", bufs=1) as wp, \
                 tc.tile_pool(name="xpool", bufs=2) as xp, \
                 tc.tile_pool(name="bias", bufs=1) as bp, \
                 tc.tile_pool(name="ptmp", bufs=3) as pt, \
                 tc.tile_pool(name="ppsum", bufs=2, space="PSUM") as pps:
                bq_t = bp.tile([P, DC], F32, tag="bq")
                bk_t = bp.tile([P, DC], F32, tag="bk")
                bv_row = bp.tile([1, GE], F32R, tag="bv")
                nc.sync.dma_start(bq_t[:], bqk[0])
                nc.sync.dma_start(bk_t[:], bqk[1])
                nc.sync.dma_start(bv_row[:], bvr)

                for which, (x_ap, w_ap, b_t, st) in enumerate(
                    [(xq, wq, bq_t, qs), (xk, wk, bk_t, ks)]
                ):
                    w_sb = wp.tile([P, EC, GE], F32R, tag=f"w{which}")
                    nc.sync.dma_start(
                        w_sb[:], w_ap.rearrange("(eo p) g -> p eo g", p=P)
                    )
                    for tb in range(TB):
                        x_sb = xp.tile([P, EC, 512], F32R, tag="xslab")
                        nc.sync.dma_start(
                            x_sb[:],
                            x_ap[:, ts(tb, 512)].rearrange("(eo p) t -> p eo t", p=P),
                        )
                        for dc in range(DC):
                            ps_t = pps.tile([P, 512], F32, tag="projps")
                            for e in range(EC):
                                nc.tensor.matmul(
                                    ps_t[:],
                                    w_sb[:, e, ts(dc, P)],
                                    x_sb[:, e, :],
                                    start=(e == 0),
                                    stop=(e == EC - 1),
                                )
                            o_t = pt.tile([P, 512], F32R, tag="projout")
                            nc.vector.tensor_scalar_add(
                                o_t[:], ps_t[:], b_t[:, dc : dc + 1]
                            )
                            nc.sync.dma_start(st[dc, :, ts(tb, 512)], o_t[:])

                # vp: natural [tok, d] layout, plus ones column
                w_sb = wp.tile([P, EC, GE], F32R, tag="wv")
                nc.sync.dma_start(w_sb[:], wv.rearrange("(eo p) g -> p eo g", p=P))
                onescol = bp.tile([P, KT], F32, tag="onescol")
                nc.gpsimd.memset(onescol[:], 1.0)
                nc.vector.tensor_copy(
                    vp[:, :, :, D : D + 1],
                    onescol[:, :, None, None].to_broadcast([P, KT, 8, 1]),
                )
                for tb in range(TB):
                    x_sb = xp.tile([P, EC, 512], F32R, tag="xslab")
                    nc.sync.dma_start(
                        x_sb[:],
                        xv[:, ts(tb, 512)].rearrange("(eo p) t -> p eo t", p=P),
                    )
                    for j in range(4):
                        c = tb * 4 + j
                        ps_t = pps.tile([P, GE], F32, tag="vps")
                        for e in range(EC):
                            nc.tensor.matmul(
                                ps_t[:],
                                x_sb[:, e, ts(j, P)],
                                w_sb[:, e, :],
                                start=(e == 0),
                                stop=False,
                            )
                        nc.tensor.matmul(
                            ps_t[:], ones[:, :P], bv_row[:], start=False, stop=True
                        )
                        nc.vector.tensor_copy(
                            vp[:, c, :, 0:D],
                            ps_t.rearrange("p (h d) -> p h d", d=D),
                        )

            # ---------------- phase 2: attention ----------------
            with tc.tile_pool(name="kq", bufs=2) as kqp, \
                 tc.tile_pool(name="expp", bufs=1) as ep, \
                 tc.tile_pool(name="dtmp", bufs=4) as dt_pool, \
                 tc.tile_pool(name="spsum", bufs=4, space="PSUM") as sps, \
                 tc.tile_pool(name="opsum", bufs=2, space="PSUM") as ops, \
                 tc.tile_pool(name="rpsum", bufs=2, space="PSUM") as rps:
                for pr in range(DC):
                    kp_t = kqp.tile([P, L], F32R, tag="kp")
                    qp_t = kqp.tile([P, L], F32R, tag="qp")
                    nc.sync.dma_start(kp_t[:], ks[pr])
                    nc.sync.dma_start(qp_t[:], qs[pr])
                    for qb in range(TB):
                        exp_t = [
                            ep.tile([P, KT, 512], F32R, tag=f"exp{i}", name=f"exp{i}")
                            for i in range(2)
                        ]
                        for kt in range(KT):
                            for i in range(2):
                                ps_s = sps.tile([P, 512], F32, tag="sc")
                                nc.tensor.matmul(
                                    ps_s[:],
                                    kp_t[ds(64 * i, 64), ts(kt, P)],
                                    qp_t[ds(64 * i, 64), ts(qb, 512)],
                                    start=True,
                                    stop=True,
                                    tile_position=(64 * i, 0),
                                )
                                nc.scalar.activation(
                                    exp_t[i][:, kt, :],
                                    ps_s[:],
                                    mybir.ActivationFunctionType.Exp,
                                    scale=float(1.0 / 32.0),
                                )
                        for i in range(2):
                            h = 2 * pr + i
                            ps_o = ops.tile([P, 512], F32, tag="ov")
                            for kt in range(KT):
                                nc.tensor.matmul(
                                    ps_o[0 : D + 1, :],
                                    vp[:, kt, h, :],
                                    exp_t[i][:, kt, :],
                                    start=(kt == 0),
                                    stop=(kt == KT - 1),
                                )
                            dinv = dt_pool.tile([1, 512], F32R, tag="dinv")
                            with nc.allow_low_precision(
                                reason="f32r denominator reciprocal"
                            ):
                                nc.vector.reciprocal(dinv[:], ps_o[D : D + 1, :])
                            ps_r = rps.tile([P, 512], F32, tag="rep")
                            nc.tensor.matmul(
                                ps_r[0:D, :], ones[:, :D], dinv[:],
                                start=True, stop=True,
                            )
                            rep_sb = dt_pool.tile([D, 512], F32R, tag="repsb")
                            nc.vector.tensor_copy(rep_sb[:], ps_r[0:D, :])
                            nc.vector.tensor_tensor(
                                ao[ds(D * i, D), pr, ts(qb, 512)],
                                ps_o[0:D, :],
                                rep_sb[:],
                                mybir.AluOpType.mult,
                            )

            # ---------------- phase 3: output projection ----------------
            with tc.tile_pool(name="wopool", bufs=1) as wop, \
                 tc.tile_pool(name="otmp", bufs=3) as ot, \
                 tc.tile_pool(name="fpsum", bufs=2, space="PSUM") as fps:
                wo_sb = wop.tile([P, DC, E], F32R)
                nc.sync.dma_start(wo_sb[:], wo.rearrange("(dc p) e -> p dc e", p=P))
                for tb in range(KT):
                    for ob in range(2):
                        ps_f = fps.tile([P, 512], F32, tag="fin")
                        for dc in range(DC):
                            nc.tensor.matmul(
                                ps_f[:],
                                ao[:, dc, ts(tb, P)],
                                wo_sb[:, dc, ts(ob, 512)],
                                start=(dc == 0),
                                stop=(dc == DC - 1),
                            )
                        o_t = ot.tile([P, 512], F32, tag="fout")
                        nc.vector.tensor_copy(o_t[:], ps_f[:])
                        nc.sync.dma_start(
                            out[ts(tb, P), ts(ob, 512)], o_t[:]
                        )

    nc.compile()
    return nc


def kernel(q, k, v, padding_mask, sequence_mask, Wq, bq, Wk, bk, Wv, bv, Wo, bo):
    # masks intentionally unused: the reference discards masked_fill results.
    if "nc" not in _CACHE:
        _CACHE["nc"] = _build()
    nc = _CACHE["nc"]

    q = np.asarray(q, np.float32)
    k = np.asarray(k, np.float32)
    v = np.asarray(v, np.float32)
    Wq = np.asarray(Wq, np.float32)
    Wk = np.asarray(Wk, np.float32)
    Wv = np.asarray(Wv, np.float32)
    Wo = np.asarray(Wo, np.float32)
    bq = np.asarray(bq, np.float32)
    bk = np.asarray(bk, np.float32)
    bv = np.asarray(bv, np.float32)
    bo = np.asarray(bo, np.float32)

    in_maps = []
    for c in range(8):
        n, g = c // 2, c % 2
        sl = slice(g * GE, (g + 1) * GE)
        bqk_arr = np.stack(
            [
                bq[sl].reshape(DC, P).T,
                bk[sl].reshape(DC, P).T,
            ]
        ).astype(np.float32)
        in_maps.append(
            {
                "xq": np.ascontiguousarray(q[n].T),
                "xk": np.ascontiguousarray(k[n].T),
                "xv": np.ascontiguousarray(v[n].T),
                "wq": np.ascontiguousarray(Wq[sl, :].T),
                "wk": np.ascontiguousarray(Wk[sl, :].T),
                "wv": np.ascontiguousarray(Wv[sl, :].T),
                "wo": np.ascontiguousarray(Wo[:, sl].T),
                "bqk": np.ascontiguousarray(bqk_arr),
                "bvr": np.ascontiguousarray(bv[sl][None, :]),
            }
        )

    import os

    trace = os.environ.get("KERNEL_TRACE") == "1"
    kw = {}
    if trace:
        kw = dict(trace=True, trace_cores=list(range(8)))
    res = run_bass_kernel_spmd(nc, in_maps, core_ids=list(range(8)), **kw)
    if trace:
        _CACHE["exec_time_ns"] = res.exec_time_ns
        _CACHE["mean_exec_time_ns"] = res.mean_exec_time_ns

    outp = np.empty((NB, L, E), np.float32)
    for n in range(NB):
        outp[n] = (
            res.results[2 * n]["out"] + res.results[2 * n + 1]["out"] + bo[None, :]
        )
    return outp


# revision 7
# speedup vs baseline: 1.4074x; 1.4074x over previous
"""Multi-head attention (N=4, L=2048, E=1024, H=16) on 8 Trainium2 cores.

Sharding: core c -> (batch n = c // 2, head-group g = c % 2).  Each core
computes, for its batch and its 8 heads (512 embed dims):
  qp_T/kp_T = (W x^T) in [d, tok] layout, vp in [tok, d] layout,
  S_T[k, q] scores with two heads packed in the 128 partitions via PE row
  tiling, exp via ACT with the 1/sqrt(1024) scale folded in, attn@v with a
  ones column appended to vp so the softmax denominator accumulates in the
  same PSUM tile, normalization via a 1-partition PE replicate matmul + DVE
  multiply, then the output projection against Wo columns of this group.
Host sums the two per-group partial outputs per batch and adds bo.

Matmul operands are fp16 (1 cycle/row on the PE at 2.4 GHz, FWL weight
loads); accumulation stays fp32 in PSUM.  fp16 keeps ~5e-4 element
precision, an order better than bf16 at the same speed.
"""

import os

import numpy as np

import concourse.bacc as bacc
import concourse.mybir as mybir
import concourse.tile as tile
from concourse.bass import ds, ts
from concourse.bass_utils import run_bass_kernel_spmd

F32 = mybir.dt.float32
F16 = mybir.dt.float16

E = 1024          # embed
H = 16            # heads (global)
D = 64            # head dim
L = 2048          # sequence length
NB = 4            # batch
GE = 512          # embed dims per head group (8 heads)
P = 128           # partitions
TB = L // 512     # 4 token blocks of 512
QB2 = L // 1024   # 2 q superblocks of 1024
EC = E // P       # 8 embed chunks
DC = GE // P      # 4 d-chunks per group == head pairs
KT = L // P       # 16 key-token chunks

_CACHE = {}


def _build():
    nc = bacc.Bacc("TRN2", debug=False, enable_asserts=False, num_devices=8)

    xq = nc.dram_tensor("xq", [E, L], F16, kind="ExternalInput").ap()
    xk = nc.dram_tensor("xk", [E, L], F16, kind="ExternalInput").ap()
    xv = nc.dram_tensor("xv", [E, L], F16, kind="ExternalInput").ap()
    wq = nc.dram_tensor("wq", [E, GE], F16, kind="ExternalInput").ap()
    wk = nc.dram_tensor("wk", [E, GE], F16, kind="ExternalInput").ap()
    wv = nc.dram_tensor("wv", [E, GE], F16, kind="ExternalInput").ap()
    wo = nc.dram_tensor("wo", [GE, E], F16, kind="ExternalInput").ap()
    bqk = nc.dram_tensor("bqk", [2, P, DC], F32, kind="ExternalInput").ap()
    bvr = nc.dram_tensor("bvr", [1, GE], F16, kind="ExternalInput").ap()
    out = nc.dram_tensor("out", [L, E], F32, kind="ExternalOutput").ap()

    with tile.TileContext(nc) as tc, \
         nc.allow_low_precision(reason="fp16 attention internals by design"):
        with tc.tile_pool(name="persist", bufs=1) as pp, \
             tc.tile_pool(name="stage", bufs=1, space="DRAM") as stage:
            # persistent SBUF
            vp = pp.tile([P, KT, 8, D + 1], F16)         # vp_aug per head
            ao = pp.tile([P, DC, L], F16)                # normalized attnout_T
            ones32 = pp.tile([1, P], F32)
            ones = pp.tile([1, P], F16)
            # DRAM staging for q/k projections, [pair, 128, L]
            qs = stage.tile([DC, P, L], F16)
            ks = stage.tile([DC, P, L], F16)

            nc.gpsimd.memset(ones32[:], 1.0)
            nc.vector.tensor_copy(ones[:], ones32[:])

            # ---------------- phase 1: projections ----------------
            with tc.tile_pool(name="wpool", bufs=1) as wp, \
                 tc.tile_pool(name="xpool", bufs=2) as xp, \
                 tc.tile_pool(name="bias", bufs=1) as bp, \
                 tc.tile_pool(name="ptmp", bufs=3) as pt, \
                 tc.tile_pool(name="ppsum", bufs=2, space="PSUM") as pps:
                bq_t = bp.tile([P, DC], F32, tag="bq")
                bk_t = bp.tile([P, DC], F32, tag="bk")
                bv_row = bp.tile([1, GE], F16, tag="bv")
                nc.sync.dma_start(bq_t[:], bqk[0])
                nc.sync.dma_start(bk_t[:], bqk[1])
                nc.sync.dma_start(bv_row[:], bvr)

                for which, (x_ap, w_ap, b_t, st) in enumerate(
                    [(xq, wq, bq_t, qs), (xk, wk, bk_t, ks)]
                ):
                    w_sb = wp.tile([P, EC, GE], F16, tag=f"w{which}")
                    nc.sync.dma_start(
                        w_sb[:], w_ap.rearrange("(eo p) g -> p eo g", p=P)
                    )
                    for tb in range(TB):
                        x_sb = xp.tile([P, EC, 512], F16, tag="xslab")
                        nc.sync.dma_start(
                            x_sb[:],
                            x_ap[:, ts(tb, 512)].rearrange("(eo p) t -> p eo t", p=P),
                        )
                        for dc in range(DC):
                            ps_t = pps.tile([P, 512], F32, tag="projps")
                            for e in range(EC):
                                nc.tensor.matmul(
                                    ps_t[:],
                                    w_sb[:, e, ts(dc, P)],
                                    x_sb[:, e, :],
                                    start=(e == 0),
                                    stop=(e == EC - 1),
                                )
                            o_t = pt.tile([P, 512], F16, tag="projout")
                            nc.vector.tensor_scalar_add(
                                o_t[:], ps_t[:], b_t[:, dc : dc + 1]
                            )
                            nc.sync.dma_start(st[dc, :, ts(tb, 512)], o_t[:])

                # vp: natural [tok, d] layout, plus ones column
                w_sb = wp.tile([P, EC, GE], F16, tag="wv")
                nc.sync.dma_start(w_sb[:], wv.rearrange("(eo p) g -> p eo g", p=P))
                onescol = bp.tile([P, KT], F32, tag="onescol")
                nc.gpsimd.memset(onescol[:], 1.0)
                nc.vector.tensor_copy(
                    vp[:, :, :, D : D + 1],
                    onescol[:, :, None, None].to_broadcast([P, KT, 8, 1]),
                )
                for tb in range(TB):
                    x_sb = xp.tile([P, EC, 512], F16, tag="xslab")
                    nc.sync.dma_start(
                        x_sb[:],
                        xv[:, ts(tb, 512)].rearrange("(eo p) t -> p eo t", p=P),
                    )
                    for j in range(4):
                        c = tb * 4 + j
                        ps_t = pps.tile([P, GE], F32, tag="vps")
                        for e in range(EC):
                            nc.tensor.matmul(
                                ps_t[:],
                                x_sb[:, e, ts(j, P)],
                                w_sb[:, e, :],
                                start=(e == 0),
                                stop=False,
                            )
                        nc.tensor.matmul(
                            ps_t[:], ones[:, :P], bv_row[:], start=False, stop=True
                        )
                        nc.vector.tensor_copy(
                            vp[:, c, :, 0:D],
                            ps_t.rearrange("p (h d) -> p h d", d=D),
                        )

            # ---------------- phase 2: attention ----------------
            with tc.tile_pool(name="kq", bufs=2) as kqp, \
                 tc.tile_pool(name="expp", bufs=1) as ep, \
                 tc.tile_pool(name="dtmp", bufs=4) as dt_pool, \
                 tc.tile_pool(name="spsum", bufs=2, space="PSUM") as sps, \
                 tc.tile_pool(name="opsum", bufs=2, space="PSUM") as ops, \
                 tc.tile_pool(name="rpsum", bufs=2, space="PSUM") as rps:
                for pr in range(DC):
                    kp_t = kqp.tile([P, L], F16, tag="kp")
                    qp_t = kqp.tile([P, L], F16, tag="qp")
                    nc.sync.dma_start(kp_t[:], ks[pr])
                    nc.sync.dma_start(qp_t[:], qs[pr])
                    for qb in range(QB2):
                        exp_t = [
                            ep.tile([P, KT, 1024], F16, tag=f"exp{i}", name=f"exp{i}")
                            for i in range(2)
                        ]
                        for kt in range(KT):
                            for i in range(2):
                                ps_s = sps.tile([P, 1024], F32, tag="sc")
                                for half in range(2):
                                    nc.tensor.matmul(
                                        ps_s[:, ts(half, 512)],
                                        kp_t[ds(64 * i, 64), ts(kt, P)],
                                        qp_t[
                                            ds(64 * i, 64),
                                            ds(qb * 1024 + half * 512, 512),
                                        ],
                                        start=True,
                                        stop=True,
                                        tile_position=(64 * i, 0),
                                    )
                                nc.scalar.activation(
                                    exp_t[i][:, kt, :],
                                    ps_s[:],
                                    mybir.ActivationFunctionType.Exp,
                                    scale=float(1.0 / 32.0),
                                )
                        for i in range(2):
                            h = 2 * pr + i
                            for half in range(2):
                                ps_o = ops.tile([P, 512], F32, tag="ov", name="ps_o")
                                for kt in range(KT):
                                    nc.tensor.matmul(
                                        ps_o[0 : D + 1, :],
                                        vp[:, kt, h, :],
                                        exp_t[i][:, kt, ts(half, 512)],
                                        start=(kt == 0),
                                        stop=(kt == KT - 1),
                                    )
                                dinv32 = dt_pool.tile([1, 512], F32, tag="dinv32")
                                nc.vector.reciprocal(dinv32[:], ps_o[D : D + 1, :])
                                dinv = dt_pool.tile([1, 512], F16, tag="dinv")
                                nc.vector.tensor_copy(dinv[:], dinv32[:])
                                ps_r = rps.tile([P, 512], F32, tag="rep")
                                nc.tensor.matmul(
                                    ps_r[0:D, :], ones[:, :D], dinv[:],
                                    start=True, stop=True,
                                )
                                rep_sb = dt_pool.tile([D, 512], F32, tag="repsb")
                                nc.vector.tensor_copy(rep_sb[:], ps_r[0:D, :])
                                nc.vector.tensor_tensor(
                                    ao[
                                        ds(D * i, D),
                                        pr,
                                        ds(qb * 1024 + half * 512, 512),
                                    ],
                                    ps_o[0:D, :],
                                    rep_sb[:],
                                    mybir.AluOpType.mult,
                                )

            # ---------------- phase 3: output projection ----------------
            with tc.tile_pool(name="wopool", bufs=1) as wop, \
                 tc.tile_pool(name="otmp", bufs=3) as ot, \
                 tc.tile_pool(name="fpsum", bufs=2, space="PSUM") as fps:
                wo_sb = wop.tile([P, DC, E], F16)
                nc.sync.dma_start(wo_sb[:], wo.rearrange("(dc p) e -> p dc e", p=P))
                for tb in range(KT):
                    for ob in range(2):
                        ps_f = fps.tile([P, 512], F32, tag="fin")
                        for dc in range(DC):
                            nc.tensor.matmul(
                                ps_f[:],
                                ao[:, dc, ts(tb, P)],
                                wo_sb[:, dc, ts(ob, 512)],
                                start=(dc == 0),
                                stop=(dc == DC - 1),
                            )
                        o_t = ot.tile([P, 512], F32, tag="fout")
                        nc.vector.tensor_copy(o_t[:], ps_f[:])
                        nc.sync.dma_start(
                            out[ts(tb, P), ts(ob, 512)], o_t[:]
                        )

    nc.compile()
    return nc


def kernel(q, k, v, padding_mask, sequence_mask, Wq, bq, Wk, bk, Wv, bv, Wo, bo):
    # masks intentionally unused: the reference discards masked_fill results.
    if "nc" not in _CACHE:
        _CACHE["nc"] = _build()
    nc = _CACHE["nc"]

    q = np.asarray(q, np.float32)
    k = np.asarray(k, np.float32)
    v = np.asarray(v, np.float32)
    Wq = np.asarray(Wq, np.float32)
    Wk = np.asarray(Wk, np.float32)
    Wv = np.asarray(Wv, np.float32)
    Wo = np.asarray(Wo, np.float32)
    bq = np.asarray(bq, np.float32)
    bk = np.asarray(bk, np.float32)
    bv = np.asarray(bv, np.float32)
    bo = np.asarray(bo, np.float32)

    in_maps = []
    for c in range(8):
        n, g = c // 2, c % 2
        sl = slice(g * GE, (g + 1) * GE)
        bqk_arr = np.stack(
            [
                bq[sl].reshape(DC, P).T,
                bk[sl].reshape(DC, P).T,
            ]
        ).astype(np.float32)
        in_maps.append(
            {
                "xq": np.ascontiguousarray(q[n].T.astype(np.float16)),
                "xk": np.ascontiguousarray(k[n].T.astype(np.float16)),
                "xv": np.ascontiguousarray(v[n].T.astype(np.float16)),
                "wq": np.ascontiguousarray(Wq[sl, :].T.astype(np.float16)),
                "wk": np.ascontiguousarray(Wk[sl, :].T.astype(np.float16)),
                "wv": np.ascontiguousarray(Wv[sl, :].T.astype(np.float16)),
                "wo": np.ascontiguousarray(Wo[:, sl].T.astype(np.float16)),
                "bqk": np.ascontiguousarray(bqk_arr),
                "bvr": np.ascontiguousarray(bv[sl][None, :].astype(np.float16)),
            }
        )

    trace = os.environ.get("KERNEL_TRACE") == "1"
    kw = {}
    if trace:
        kw = dict(trace=True, trace_cores=list(range(8)))
    res = run_bass_kernel_spmd(nc, in_maps, core_ids=list(range(8)), **kw)
    if trace:
        _CACHE["exec_time_ns"] = res.exec_time_ns
        _CACHE["mean_exec_time_ns"] = res.mean_exec_time_ns

    outp = np.empty((NB, L, E), np.float32)
    for n in range(NB):
        outp[n] = (
            res.results[2 * n]["out"] + res.results[2 * n + 1]["out"] + bo[None, :]
        )
    return outp
